# revision 2
# baseline (speedup 1.0000x reference)
"""Trainium2 Bass kernel for nn_CNN_88098369175780 — v2.

Single NEFF launch on 8 cores (head fused via one small AllGather):
  - Sequence-parallel attention, 512 q-cols/core, NO halo columns: the two
    cross-core boundary columns are exchanged through the same AllGather that
    gathers conv2 outputs, and the few conv outputs that depend on them are
    recomputed (fix-up) redundantly on every core before the FC head.
  - K/V are derived through the 16x16 Gram fold: wavP2 = Gram @ (wav*E), so
    K = (W1*Gram)(wav*E)+b1 -- the W*Gram products are computed on-chip and
    the big [T,T]-free preprocessing chain loses one matmul+elementwise hop.
  - Scores run 4-way row-tiled on the PE (groups 0/32/64/96, explicit
    tile_position for group 96).  Block A's softmax numerator uses the Act
    engine's Exp->fp8e5; block B uses a Schraudolph trick on the DVE:
    psum already holds 4*log2e*(s-shift); DVE adds 60, clamps at 0, and
    converts to int8, which bitcast as e5m2 IS 2^((b-60)/4) ~ exp(s-shift).
  - A@V runs in DoubleRow fp8 with a widened lhs: V columns 32:48 are ones,
    so U rows 32:48 hold the softmax denominator replicated 16-wide; the
    reciprocal runs as one fast [16,512] custom-DVE op and no partition
    broadcasts are needed anywhere in the softmax-S epilogue.
"""
import contextlib
import ctypes
import os
import sys
import types

import numpy as np

os.environ.setdefault("NEURON_RT_RESET_CORES", "1")

for _p in ('/root/.axon_site', '/root/.axon_site/_ro/trn_rl_repo',
           '/root/.axon_site/_ro/pypackages', '/opt/trn_rl_repo'):
    if os.path.isdir(_p) and _p not in sys.path:
        sys.path.append(_p)

import ml_dtypes
import concourse.bacc as bacc
import concourse.tile as tile
import concourse.mybir as mybir
from concourse.bass_utils import run_bass_kernel_spmd

f32 = mybir.dt.float32
bf16 = mybir.dt.bfloat16
int8 = mybir.dt.int8
f8e4 = mybir.dt.float8e4
f8e5 = mybir.dt.float8e5
AF = mybir.ActivationFunctionType
ALU = mybir.AluOpType
DR = mybir.MatmulPerfMode.DoubleRow
BF = ml_dtypes.bfloat16
E4 = ml_dtypes.float8_e4m3fn
E5 = ml_dtypes.float8_e5m2

T = 4096
NC = 8
KLOG = 4.0 / np.log(2.0)   # 4*log2(e): Schraudolph scale folded into Q_B

USE_GRP96 = True       # P1: 4th PE row tile
USE_SCHRAUD = False    # int8 exp hurt accuracy (3.4e-2); Act exp both blocks
USE_UB_COL64 = False   # DR matmul dst must start at partition 0 (ISA)
USE_FREEBCAST = True   # P5: stride-0 free-dim operand for wavPT build
USE_DMABCAST = True    # P4: partition-broadcast DMA for wavM build


# ---------------------------------------------------------------- NTFF shim
def _install_ntff_shim():
    name = "antenv.axon_hooks"
    if name in sys.modules:
        return
    so_path = "/opt/axon/libaxon_pjrt.so"
    hook = None
    if os.path.exists(so_path):
        lib = ctypes.CDLL(so_path)
        if hasattr(lib, "axon_start_nrt_profile"):
            lib.axon_start_nrt_profile.argtypes = [
                ctypes.POINTER(ctypes.c_int64), ctypes.c_size_t]
            lib.axon_start_nrt_profile.restype = ctypes.c_int64
            lib.axon_stop_nrt_profile.argtypes = [ctypes.c_char_p]
            lib.axon_stop_nrt_profile.restype = ctypes.c_int64

            @contextlib.contextmanager
            def _hook(output_dir, device_ids):
                import jax
                jax.devices()
                def _start():
                    if device_ids:
                        ids = (ctypes.c_int64 * len(device_ids))(*device_ids)
                        return lib.axon_start_nrt_profile(ids, len(device_ids))
                    return lib.axon_start_nrt_profile(None, 0)
                rc = _start()
                if rc != 0:
                    import tempfile
                    lib.axon_stop_nrt_profile(tempfile.mkdtemp().encode())
                    rc = _start()
                if rc != 0:
                    sys.stderr.write(f"WARN: nrt profile unavailable rc={rc}\n")
                    yield
                    return
                try:
                    yield
                finally:
                    try:
                        n = lib.axon_stop_nrt_profile(str(output_dir).encode())
                        if n < 0:
                            sys.stderr.write(f"WARN: stop_nrt_profile rc={n}\n")
                    except Exception:
                        pass
            hook = _hook
    mod = types.ModuleType(name)
    mod._hook = hook
    mod.set_axon_ntff_profile_hook = lambda h: setattr(mod, "_hook", h)
    mod.get_axon_ntff_profile_hook = lambda: mod._hook
    sys.modules[name] = mod


_install_ntff_shim()


# ------------------------------------------------------------- host consts
def build_consts(x, cm1_W, cm1_b, cm2_W, cm2_b, cw0, cw1, cw2, cw3, cb,
                 fc1_W, fc1_b, fc2_W, fc2_b):
    F = np.float32
    x = np.asarray(x, F)
    eeg2 = np.ascontiguousarray(x[0, 0, 1:-1, :]).astype(F)
    wavA = np.ascontiguousarray(x[0, 0, 0, :]).astype(F)
    wavB = np.ascontiguousarray(x[0, 0, -1, :]).astype(F)
    cm1_W = np.asarray(cm1_W, F); cm1_b = np.asarray(cm1_b, F)
    cm2_W = np.asarray(cm2_W, F); cm2_b = np.asarray(cm2_b, F)
    cw0 = np.asarray(cw0, F); cw1 = np.asarray(cw1, F)
    cw2 = np.asarray(cw2, F); cw3 = np.asarray(cw3, F); cb = np.asarray(cb, F)
    fc1_W = np.asarray(fc1_W, F); fc1_b = np.asarray(fc1_b, F)
    fc2_W = np.asarray(fc2_W, F); fc2_b = np.asarray(fc2_b, F)

    c = {}
    E_aug = np.concatenate([eeg2, np.ones((1, T), F)], 0)          # [17, T]
    c['EA'] = E_aug.astype(BF)
    wav3 = np.stack([wavA, wavB, np.ones(T, F)], 0)
    c['WAV'] = wav3.astype(BF)

    et = np.transpose(eeg2.reshape(16, 32, 128), (2, 1, 0))        # [128,32,16]
    wa = wavA.reshape(32, 128).T                                   # [128, 32]
    wb = wavB.reshape(32, 128).T
    if USE_FREEBCAST:
        c['ET'] = np.concatenate(
            [et.reshape(128, 512), wa, wb], 1).astype(BF)          # [128, 576]
    else:
        ET_dup = np.concatenate([et, et], axis=2).reshape(128, 1024)
        wav_exp = np.concatenate(
            [np.repeat(wa[:, :, None], 16, 2),
             np.repeat(wb[:, :, None], 16, 2)], axis=2).reshape(128, 1024)
        c['ET'] = np.concatenate([ET_dup, wav_exp], 1).astype(BF)  # [128,2048]

    kB = KLOG if USE_SCHRAUD else 1.0
    # CW [49, 450]: 0:113 lhsK-const | 113:241 rhsV-const | 241:354 lhsQ |
    #   354:370 W3A | 370:386 W3B | 386:402 W1AT | 402:418 W1BT |
    #   418:434 W2AT | 434:450 W2BT
    CW = np.zeros((49, 450), F)
    # lhsK const: bias rows + (+6) shift cols (shift survives the 0..6 clamp;
    # QTmix shift rows hold the NEGATED k-scaled sumQ)
    CW[16, 0:16] = cm1_b[1]; CW[16, 16] = 6.0
    CW[16, 32:48] = cm1_b[1]; CW[16, 48] = 6.0
    CW[48, 64:80] = cm2_b[1]; CW[48, 80] = 6.0
    CW[48, 96:112] = cm2_b[1]; CW[48, 112] = 6.0
    # rhsV const [49, 128] at cols 113:241: A cols 0:64, B cols 64:128
    # (V at +0:16, ones at +32:48 -> U rows 32:48 = replicated denominator)
    CW[16, 113 + 0:113 + 16] = cm1_b[2]
    CW[16, 113 + 32:113 + 48] = 1.0
    CW[48, 113 + 64:113 + 80] = cm2_b[2]
    CW[16, 113 + 96:113 + 112] = 1.0
    # lhsQ [17, 113] at cols 241:354
    CW[0:16, 241 + 0:241 + 16] = cm1_W[0].T
    CW[0:16, 241 + 32:241 + 48] = cm1_W[0].T
    CW[0:16, 241 + 64:241 + 80] = kB * cm2_W[0].T
    CW[0:16, 241 + 96:241 + 112] = kB * cm2_W[0].T
    CW[16, 241 + 0:241 + 16] = cm1_b[0]
    CW[16, 241 + 32:241 + 48] = cm1_b[0]
    CW[16, 241 + 64:241 + 80] = kB * cm2_b[0]
    CW[16, 241 + 96:241 + 112] = kB * cm2_b[0]
    # W3 (aug with bias row), W-fold rhs blocks
    CW[0:17, 354:370] = np.concatenate([cm1_W[3].T, cm1_b[3][None, :]], 0)
    CW[0:17, 370:386] = np.concatenate([cm2_W[3].T, cm2_b[3][None, :]], 0)
    CW[0:16, 386:402] = cm1_W[1].T
    CW[0:16, 402:418] = cm2_W[1].T
    CW[0:16, 418:434] = cm1_W[2].T
    CW[0:16, 434:450] = cm2_W[2].T
    c['CW'] = CW.astype(BF)

    # CB [128, 18]: col 0 = -1 at rows 0:16 and 64:80 (negated sumQ lhs);
    # cols 2:18 = ones16x16 at rows 0:16 (replicated softmax-S denominator)
    CBv = np.zeros((128, 18), F)
    CBv[0:16, 0] = -1.0
    CBv[64:80, 0] = -1.0
    CBv[0:16, 2:18] = 1.0
    c['CB'] = CBv.astype(BF)

    if not USE_DMABCAST:
        wm = np.zeros((49, T), F)
        wm[0:16] = wavA[None, :]; wm[16] = 1.0
        wm[32:48] = wavB[None, :]; wm[48] = 1.0
        c['WAVM'] = wm.astype(BF)

    # fp8 consts
    c['F8'] = np.ones((1, 512), np.float64).astype(E4)
    f8e5c = np.zeros((1, 64), np.float64)
    f8e5c[0, 32:48] = 2.0 ** -14
    c['F8E5'] = f8e5c.astype(E5)

    # fix-up const FXC [49, 144]: per window c (core c), 18 cols of the
    # y48 slice [512c+496 : 512c+514): rows 0:16 = E, row 48 = 1.
    fxc = np.zeros((49, 144), F)
    for ci in range(NC):
        lo = 512 * ci + 496
        n = min(18, T - lo)
        fxc[0:16, 18 * ci:18 * ci + n] = eeg2[:, lo:lo + n]
        fxc[48, 18 * ci:18 * ci + 18] = 1.0
    c['FXC'] = fxc.astype(BF)

    # conv weight unfolds (identical to baseline)
    def y48row(origH):
        if 16 <= origH < 32:
            return origH - 16
        if origH < 16:
            return origH + 16
        return origH
    c0 = np.zeros((3, 49, 120), F)
    for dw in range(3):
        for cch in range(5):
            for h in range(24):
                m = cch * 24 + h
                for dh in range(2):
                    c0[dw, y48row(2 * h + dh), m] += cw0[cch, 0, dh, dw]
                if dw == 0:
                    c0[dw, 48, m] += cb[0][cch]
    c1 = np.zeros((4, 121, 60), F)
    for dw in range(4):
        for cch in range(5):
            for h in range(12):
                m = cch * 12 + h
                for cin in range(5):
                    for dh in range(2):
                        c1[dw, cin * 24 + 2 * h + dh, m] += cw1[cch, cin, dh, dw]
                if dw == 0:
                    c1[dw, 120, m] += cb[1][cch]
    c2 = np.zeros((4, 61, 30), F)
    for dw in range(4):
        for cch in range(5):
            for h in range(6):
                m = cch * 6 + h
                for cin in range(5):
                    for dh in range(2):
                        c2[dw, cin * 12 + 2 * h + dh, m] += cw2[cch, cin, dh, dw]
                if dw == 0:
                    c2[dw, 60, m] += cb[2][cch]
    cvw = np.zeros((121, 720), F)
    for dw in range(3):
        cvw[0:49, 120 * dw:120 * dw + 120] = c0[dw]
    for dw in range(4):
        cvw[0:121, 360 + 60 * dw:360 + 60 * dw + 60] = c1[dw]
    for dw in range(4):
        cvw[0:61, 600 + 30 * dw:600 + 30 * dw + 30] = c2[dw]
    c['CONVW'] = cvw.astype(BF)

    # head consts (identical to baseline)
    c3 = np.zeros((4, 31, 15), F)
    for dw in range(4):
        for cch in range(5):
            for h in range(3):
                m = cch * 3 + h
                for cin in range(5):
                    for dh in range(2):
                        c3[dw, cin * 6 + 2 * h + dh, m] += cw3[cch, cin, dh, dw]
                if dw == 0:
                    c3[dw, 30, m] += cb[3][cch]
    hw = np.zeros((31, 77), F)
    for dw in range(4):
        hw[:, 15 * dw:15 * dw + 15] = c3[dw]
    hw[:, 60:75] = np.concatenate([fc1_W.T, fc1_b[None, :]], 0)
    w_d = np.stack([fc2_W[0] - fc2_W[1], fc2_W[1] - fc2_W[0]], 1)
    b_d = np.array([fc2_b[0] - fc2_b[1], fc2_b[1] - fc2_b[0]], F)
    hw[0:16, 75:77] = np.concatenate([w_d, b_d[None, :]], 0)
    c['HEADW'] = hw.astype(BF)
    c['HONES'] = np.ones((1, 256), F).astype(BF)

    # per-core E slice [17, 514]
    cxs = np.zeros((NC, 17, 514), F)
    for ci in range(NC):
        n = min(514, T - 512 * ci)
        cxs[ci, :, 0:n] = E_aug[:, 512 * ci:512 * ci + n]
    c['CX'] = cxs.astype(BF)
    return c


# ---------------------------------------------------------------- kernel
def _build():
    nc = bacc.Bacc("TRN2", target_bir_lowering=False, debug=False,
                   num_devices=NC)
    dt = nc.dram_tensor
    et_w = 576 if USE_FREEBCAST else 2048
    a = {
        'ET':    dt('ET',    [128, et_w], bf16, kind="ExternalInput").ap(),
        'EA':    dt('EA',    [17, T],     bf16, kind="ExternalInput").ap(),
        'WAV':   dt('WAV',   [3, T],      bf16, kind="ExternalInput").ap(),
        'CW':    dt('CW',    [49, 450],   bf16, kind="ExternalInput").ap(),
        'CB':    dt('CB',    [128, 18],   bf16, kind="ExternalInput").ap(),
        'CONVW': dt('CONVW', [121, 720],  bf16, kind="ExternalInput").ap(),
        'F8':    dt('F8',    [1, 512],    f8e4, kind="ExternalInput").ap(),
        'F8E5':  dt('F8E5',  [1, 64],     f8e5, kind="ExternalInput").ap(),
        'CX':    dt('CX',    [17, 514],   bf16, kind="ExternalInput").ap(),
        'stg':   dt('stg',   [62, 32],    bf16, kind="ExternalOutput").ap(),
    }
    if not USE_DMABCAST:
        a['WAVM'] = dt('WAVM', [49, T], bf16, kind="ExternalInput").ap()

    with tile.TileContext(nc) as tc:
        with tc.tile_pool(name="const", bufs=1) as cp, \
             tc.tile_pool(name="work", bufs=2) as wp, \
             tc.tile_pool(name="exps", bufs=2) as ep, \
             tc.tile_pool(name="psumP", bufs=2, space="PSUM") as psP, \
             tc.tile_pool(name="psumU", bufs=1, space="PSUM") as psU, \
             tc.tile_pool(name="psumS", bufs=1, space="PSUM") as psS, \
             tc.tile_pool(name="psumC", bufs=1, space="PSUM") as psC, \
             tc.tile_pool(name="dram", bufs=1, space="DRAM") as dp:

            # ---------------- input loads (spread across queues)
            ET = cp.tile([128, et_w], bf16, tag="ET")
            EAB = cp.tile([49, T], bf16, tag="EAB")
            wavM = cp.tile([49, T], bf16, tag="wavM")
            CWt = cp.tile([49, 450], bf16, tag="CWt")
            CBt = cp.tile([128, 18], bf16, tag="CBt")
            lhsK = cp.tile([49, 113], bf16, tag="lhsK")
            rhsV = cp.tile([49, 128], bf16, tag="rhsV")
            CONVW = cp.tile([121, 720], bf16, tag="CONVW")
            F8 = cp.tile([1, 512], f8e4, tag="F8")
            F8E5 = cp.tile([1, 64], f8e5, tag="F8E5")
            CX = cp.tile([17, 514], bf16, tag="CX")

            nc.sync.dma_start(ET[:], a['ET'][:])
            nc.sync.dma_start(EAB[0:17, :], a['EA'][:])
            nc.sync.dma_start(EAB[32:49, :], a['EA'][:])
            if USE_DMABCAST:
                nc.scalar.dma_start(wavM[0:16, :],
                                    a['WAV'][0:1, :].to_broadcast((16, T)))
                nc.scalar.dma_start(wavM[16:17, :], a['WAV'][2:3, :])
                nc.scalar.dma_start(wavM[32:48, :],
                                    a['WAV'][1:2, :].to_broadcast((16, T)))
                nc.scalar.dma_start(wavM[48:49, :], a['WAV'][2:3, :])
            else:
                nc.scalar.dma_start(wavM[0:17, :], a['WAVM'][0:17, :])
                nc.scalar.dma_start(wavM[32:49, :], a['WAVM'][32:49, :])
            nc.gpsimd.dma_start(CWt[:], a['CW'][:])
            nc.gpsimd.dma_start(CBt[:], a['CB'][:])
            nc.gpsimd.dma_start(lhsK[:], a['CW'][:, 0:113])
            nc.gpsimd.dma_start(rhsV[:], a['CW'][:, 113:241])
            nc.gpsimd.dma_start(CX[:], a['CX'][:])
            nc.sync.dma_start(CONVW[:], a['CONVW'][:])
            nc.scalar.dma_start(F8[:], a['F8'][:])
            nc.scalar.dma_start(F8E5[:], a['F8E5'][:])

            lhsQ = CWt[0:17, 241:354]
            W3A = CWt[0:17, 354:370]
            W3B = CWt[0:17, 370:386]
            W1AT = CWt[0:16, 386:402]
            W1BT = CWt[0:16, 402:418]
            W2AT = CWt[0:16, 418:434]
            W2BT = CWt[0:16, 434:450]
            negO = CBt[:, 0:1]
            ones16c = CBt[0:16, 2:18]
            onesrow8 = F8[0:1, 0:512]
            epsW = F8E5[0:1, 0:64]
            E_sl = CX[0:17, 0:514]

            # ---------------- wavP = wav * E  (two aligned DVE mults)
            wavP = cp.tile([49, T], bf16, tag="wavP")
            nc.vector.memset(wavP[0:49, :], 0.0)
            nc.vector.tensor_tensor(wavP[0:17, :], EAB[0:17, :], wavM[0:17, :],
                                    op=ALU.mult)
            nc.vector.tensor_tensor(wavP[32:49, :], EAB[32:49, :],
                                    wavM[32:49, :], op=ALU.mult)

            # ---------------- Gram matrices (contraction over T on PE)
            wavPT = cp.tile([128, 1024], bf16, tag="wavPT")
            if USE_FREEBCAST:
                etv = ET[:, 0:512].rearrange("p (c f) -> p c f", f=16)
                wpt = wavPT[:].rearrange("p (c f) -> p c f", f=32)
                nc.vector.tensor_tensor(
                    wpt[:, :, 0:16], etv,
                    ET[:, 512:544].unsqueeze(2).to_broadcast((128, 32, 16)),
                    op=ALU.mult)
                nc.vector.tensor_tensor(
                    wpt[:, :, 16:32], etv,
                    ET[:, 544:576].unsqueeze(2).to_broadcast((128, 32, 16)),
                    op=ALU.mult)
                def et_chunk(g):
                    return ET[:, 16 * g:16 * g + 16]
            else:
                nc.vector.tensor_tensor(wavPT[:], ET[:, 0:1024],
                                        ET[:, 1024:2048], op=ALU.mult)
                def et_chunk(g):
                    return ET[:, 32 * g:32 * g + 16]

            gps = psS.tile([16, 32], f32, tag="S")
            for g in range(32):
                nc.tensor.matmul(gps[:], et_chunk(g),
                                 wavPT[:, 32 * g:32 * g + 32],
                                 start=(g == 0), stop=(g == 31))
            GA = wp.tile([16, 16], bf16, tag="GA")
            GB = wp.tile([16, 16], bf16, tag="GB")
            nc.vector.tensor_copy(GA[:], gps[:, 0:16])
            nc.vector.tensor_copy(GB[:], gps[:, 16:32])

            # ---------------- W*Gram folds -> dynamic parts of lhsK / rhsV
            fA = psS.tile([48, 32], f32, tag="S")
            nc.tensor.matmul(fA[0:16, 0:16], GA[:], W1AT, start=True, stop=True)
            nc.tensor.matmul(fA[0:16, 16:32], GA[:], W2AT, start=True, stop=True)
            nc.tensor.matmul(fA[32:48, 0:16], GB[:], W1BT, start=True, stop=True,
                             tile_position=(0, 32))
            nc.tensor.matmul(fA[32:48, 16:32], GB[:], W2BT, start=True, stop=True,
                             tile_position=(0, 32))
            nc.vector.tensor_copy(lhsK[0:16, 0:16], fA[0:16, 0:16])
            nc.vector.tensor_copy(lhsK[0:16, 32:48], fA[0:16, 0:16])
            nc.vector.tensor_copy(lhsK[32:48, 64:80], fA[32:48, 0:16])
            nc.vector.tensor_copy(lhsK[32:48, 96:112], fA[32:48, 0:16])
            nc.vector.tensor_copy(rhsV[0:16, 0:16], fA[0:16, 16:32])
            nc.vector.tensor_copy(rhsV[32:48, 64:80], fA[32:48, 16:32])

            # ---------------- Q + negated sumQ rows
            QTmix = cp.tile([128, 512], bf16, tag="QTmix")
            qp = psP.tile([128, 1024], f32, tag="P")
            nc.tensor.matmul(qp[0:113, 0:512], lhsQ, E_sl[:, 0:512],
                             start=True, stop=True)
            nc.vector.tensor_scalar(QTmix[0:113, :], qp[0:113, 0:512],
                                    0.0, 6.0, ALU.max, ALU.min)
            sqA = psS.tile([1, 512], f32, tag="S")
            nc.tensor.matmul(sqA[:], negO[0:16, :], QTmix[0:16, :],
                             start=True, stop=True)
            sqAb = wp.tile([1, 512], bf16, tag="sqAb")
            nc.vector.tensor_copy(sqAb[:], sqA[:])
            sqB = psS.tile([1, 512], f32, tag="S")
            nc.tensor.matmul(sqB[:], negO[64:80, :], QTmix[64:80, :],
                             start=True, stop=True)
            sqBb = wp.tile([1, 512], bf16, tag="sqBb")
            nc.vector.tensor_copy(sqBb[:], sqB[:])
            nc.sync.dma_start(QTmix[16:17, :], sqAb[:])
            nc.sync.dma_start(QTmix[48:49, :], sqAb[:])
            nc.sync.dma_start(QTmix[80:81, :], sqBb[:])
            nc.sync.dma_start(QTmix[112:113, :], sqBb[:])

            # ---------------- K (Gram-folded), 4 row-tile copies
            KTmix = cp.tile([128, T], bf16, tag="KTmix")
            for j in range(8):
                kps = psP.tile([128, 1024], f32, tag="P")
                nc.tensor.matmul(kps[0:113, 0:512], lhsK[:],
                                 wavP[:, 512 * j:512 * j + 512],
                                 start=True, stop=True)
                nc.vector.tensor_scalar(KTmix[0:113, 512 * j:512 * j + 512],
                                        kps[0:113, 0:512], 0.0, 6.0,
                                        ALU.max, ALU.min)

            # ---------------- V (fp8 e4m3, widened ones cols 32:48/80:96)
            Vt = cp.tile([128, 32 * 128], f8e4, tag="Vt")
            for q in range(8):
                vps = psP.tile([128, 1024], f32, tag="P")
                for kk in range(2):
                    g = 2 * q + kk
                    nc.tensor.matmul(vps[:, 512 * kk:512 * kk + 128],
                                     wavP[:, 128 * (4 * q + 2 * kk):
                                          128 * (4 * q + 2 * kk) + 128],
                                     rhsV[:], start=True, stop=True)
                    nc.tensor.matmul(vps[:, 512 * kk + 128:512 * kk + 256],
                                     wavP[:, 128 * (4 * q + 2 * kk + 1):
                                          128 * (4 * q + 2 * kk + 1) + 128],
                                     rhsV[:], start=True, stop=True)
                    nc.vector.tensor_scalar(
                        Vt[:, 512 * q + 256 * kk:512 * q + 256 * kk + 256],
                        vps[:, 512 * kk:512 * kk + 256], 0.0, 6.0,
                        ALU.max, ALU.min)

            def vt_pair(p, bi):
                return Vt[:].rearrange("p (g f) -> p g f", f=128)[
                    :, 2 * p:2 * p + 2, 64 * bi:64 * bi + 64]

            # ---------------- y48 assembly target
            y48 = cp.tile([49, 514], bf16, tag="y48")
            nc.vector.memset(y48[0:48, 512:514], 0.0)
            nc.sync.dma_start(y48[0:16, :], a['CX'][0:16, :])
            nc.sync.dma_start(y48[48:49, :], a['CX'][16:17, :])

            # ---------------- pair loop
            UA = psU.tile([64, 512], f32, tag="UA")
            UBt = psU.tile([64, 512], f32, tag="UB")
            UB = UBt[0:64, :]
            nc.tensor.matmul(UA[:], epsW, onesrow8, start=True, stop=False)
            nc.tensor.matmul(UB, epsW, onesrow8, start=True, stop=False)

            grpB1 = 96 if USE_GRP96 else 64
            exq = {}

            def emit_scores(bi, p, pair):
                for par in (0, 1):
                    g = 2 * p + par
                    lo = (0, 32)[par] if bi == 0 else (64, grpB1)[par]
                    nc.tensor.matmul(pair[:, 512 * par:512 * par + 512],
                                     KTmix[lo:lo + 17, 128 * g:128 * g + 128],
                                     QTmix[lo:lo + 17, 0:512],
                                     start=True, stop=True,
                                     tile_position=(lo, 0))

            def emit_ex(bi, p, pair):
                if bi == 0 or not USE_SCHRAUD:
                    ex = ep.tile([128, 1024], f8e5, tag=f"ex{bi}")
                    nc.scalar.activation(ex[:], pair[:], AF.Exp)
                else:
                    ex = ep.tile([128, 1024], int8, tag="ex1")
                    nc.vector.tensor_scalar(ex[:], pair[:], 60.0, 0.0,
                                            ALU.add, ALU.max)
                exq[(bi, p)] = ex

            def emit_u(bi, p):
                ex = exq.pop((bi, p))
                exv = ex[:]
                if bi == 1 and USE_SCHRAUD:
                    exv = exv.bitcast(f8e5)
                nc.tensor.matmul(UA[:] if bi == 0 else UB, vt_pair(p, bi),
                                 exv.rearrange("p (g t) -> p g t", g=2),
                                 start=False, stop=(p == 15), perf_mode=DR)

            for p in range(17):
                if p < 16:
                    pairA = psP.tile([128, 1024], f32, tag="P")
                    emit_scores(0, p, pairA)
                    pairB = psP.tile([128, 1024], f32, tag="P")
                    emit_scores(1, p, pairB)
                    emit_ex(0, p, pairA)
                    emit_ex(1, p, pairB)
                if p >= 1:
                    emit_u(0, p - 1)
                    emit_u(1, p - 1)

            # ---------------- softmax-S epilogue + W3 (per block)
            def emit_z(bi):
                U = UA[:] if bi == 0 else UB
                W3 = W3A if bi == 0 else W3B
                rUb = wp.tile([48, 512], f32, tag=f"rUb{bi}")
                nc.vector.reciprocal_approx_fast(rUb[32:48, :], U[32:48, :])
                AVn = wp.tile([16, 512], f32, tag=f"AVn{bi}")
                nc.vector.tensor_tensor(AVn[:], U[0:16, :], rUb[32:48, :],
                                        op=ALU.mult)
                Z = wp.tile([17, 512], bf16, tag=f"Z{bi}")
                nc.scalar.activation(Z[0:16, :], AVn[:], AF.Exp)
                dn = psS.tile([16, 512], f32, tag="S")
                nc.tensor.matmul(dn[:], ones16c, Z[0:16, :],
                                 start=True, stop=True)
                dnb = wp.tile([1, 512], bf16, tag=f"dnb{bi}")
                nc.scalar.activation(dnb[:], dn[0:1, :], AF.Copy)
                nc.sync.dma_start(Z[16:17, :], dnb[:])
                rd = wp.tile([16, 512], f32, tag=f"rd{bi}")
                nc.vector.reciprocal_approx_fast(rd[:], dn[:])
                o31 = psS.tile([16, 512], f32, tag="S")
                nc.tensor.matmul(o31[:], W3, Z[:], start=True, stop=True)
                wavm = wp.tile([16, 512], f32, tag=f"wavm{bi}")
                nc.vector.tensor_tensor(wavm[:], o31[:], rd[:], op=ALU.mult)
                wavc = wp.tile([16, 512], bf16, tag=f"wavc{bi}")
                nc.vector.tensor_scalar(wavc[:], wavm[:], 0.0, 6.0,
                                        ALU.max, ALU.min)
                nc.sync.dma_start(y48[16 + 16 * bi:32 + 16 * bi, 0:512],
                                  wavc[:])

            emit_z(0)
            c0w = [CONVW[0:49, 120 * dw:120 * dw + 120] for dw in range(3)]
            c1w = [CONVW[0:121, 360 + 60 * dw:360 + 60 * dw + 60]
                   for dw in range(4)]
            c2w = [CONVW[0:61, 600 + 30 * dw:600 + 30 * dw + 30]
                   for dw in range(4)]
            c0ps = psC.tile([120, 512], f32, tag="C")
            for dw in range(3):
                nc.tensor.matmul(c0ps[:], c0w[dw][0:32, :],
                                 y48[0:32, dw:dw + 512],
                                 start=(dw == 0), stop=False)
            emit_z(1)
            for dw in range(3):
                nc.tensor.matmul(c0ps[:], c0w[dw][32:49, :],
                                 y48[32:49, dw:dw + 512],
                                 start=False, stop=(dw == 2))

            # ---------------- conv stack on the local 512 columns
            y0 = cp.tile([121, 512], bf16, tag="y0")
            nc.vector.tensor_scalar(y0[0:120, :], c0ps[:], 0.0, 6.0,
                                    ALU.max, ALU.min)
            nc.sync.dma_start(y0[120:121, 0:512], a['WAV'][2:3, 0:512])
            y1 = cp.tile([61, 128], bf16, tag="y1")
            c1ps = psC.tile([60, 128], f32, tag="C")
            for dw in range(4):
                rhs = y0[:].rearrange("p (n s) -> p n s", s=4)[:, :, dw]
                nc.tensor.matmul(c1ps[:], c1w[dw], rhs, start=(dw == 0),
                                 stop=(dw == 3))
            nc.vector.tensor_scalar(y1[0:60, :], c1ps[:], 0.0, 6.0,
                                    ALU.max, ALU.min)
            nc.sync.dma_start(y1[60:61, 0:128], a['WAV'][2:3, 0:128])
            c2ps = psC.tile([30, 32], f32, tag="C")
            for dw in range(4):
                rhs = y1[:].rearrange("p (n s) -> p n s", s=4)[:, :, dw]
                nc.tensor.matmul(c2ps[:], c2w[dw], rhs, start=(dw == 0),
                                 stop=(dw == 3))

            # ---------------- gather payload + AllGather
            stage = cp.tile([62, 32], bf16, tag="stage")
            nc.vector.memset(stage[0:62, :], 0.0)
            nc.vector.tensor_scalar(stage[0:30, :], c2ps[:], 0.0, 6.0,
                                    ALU.max, ALU.min)
            nc.scalar.dma_start(stage[30:62, 0:16], y48[16:48, 496:512])
            nc.scalar.dma_start(stage[30:62, 16:18], y48[16:48, 0:2])
            nc.gpsimd.dma_start(a['stg'][:], stage[:])
    nc.compile()
    return nc


_NC1 = None
_NC2 = None


def _build_head():
    nc = bacc.Bacc("TRN2", target_bir_lowering=False, debug=False,
                   num_devices=1)
    dt = nc.dram_tensor
    a = {
        'GATH':  dt('GATH',  [496, 32],  bf16, kind="ExternalInput").ap(),
        'FXC':   dt('FXC',   [49, 144],  bf16, kind="ExternalInput").ap(),
        'CONVW': dt('CONVW', [121, 720], bf16, kind="ExternalInput").ap(),
        'HEADW': dt('HEADW', [31, 77],   bf16, kind="ExternalInput").ap(),
        'HONES': dt('HONES', [1, 256],   bf16, kind="ExternalInput").ap(),
        'WAV':   dt('WAV',   [3, T],     bf16, kind="ExternalInput").ap(),
        'out':   dt('out',   [42, 2],    f32, kind="ExternalOutput").ap(),
    }
    with tile.TileContext(nc) as tc:
        with tc.tile_pool(name="sb", bufs=1) as cp, \
             tc.tile_pool(name="wk", bufs=2) as wp, \
             tc.tile_pool(name="ps", bufs=2, space="PSUM") as psC, \
             tc.tile_pool(name="dram", bufs=1, space="DRAM") as dp:
            CONVW = cp.tile([121, 720], bf16, tag="CONVW")
            HEADW = cp.tile([31, 77], bf16, tag="HEADW")
            nc.sync.dma_start(CONVW[:], a['CONVW'][:])
            nc.scalar.dma_start(HEADW[:], a['HEADW'][:])
            c0w = [CONVW[0:49, 120 * dw:120 * dw + 120] for dw in range(3)]
            c1w = [CONVW[0:121, 360 + 60 * dw:360 + 60 * dw + 60]
                   for dw in range(4)]
            c2w = [CONVW[0:61, 600 + 30 * dw:600 + 30 * dw + 30]
                   for dw in range(4)]
            go = a['GATH'][:].rearrange("(d r) c -> r d c", d=NC)
            y2a = cp.tile([31, 256], bf16, tag="y2a")
            nc.sync.dma_start(y2a[0:30, :].rearrange("p (d c) -> p d c", d=NC),
                              go[0:30, :, :])
            nc.sync.dma_start(y2a[30:31, :], a['HONES'][:])
            y48w = cp.tile([49, 144], bf16, tag="y48w")
            nc.scalar.dma_start(y48w[:], a['FXC'][:])
            y48wv = y48w[16:48, :].rearrange("p (w x) -> p w x", x=18)
            nc.scalar.dma_start(y48wv[:, :, 0:16], go[30:62, :, 0:16])
            nc.scalar.dma_start(y48wv[:, 0:7, 16:18], go[30:62, 1:8, 16:18])

            c0f = psC.tile([120, 128], f32, tag="C")
            y48wx = y48w[:].rearrange("p (w x) -> p w x", x=18)
            for dw in range(3):
                nc.tensor.matmul(c0f[:].rearrange("p (w m) -> p w m", m=16),
                                 c0w[dw][0:49, :], y48wx[:, :, dw:dw + 16],
                                 start=(dw == 0), stop=(dw == 2))
            y0f = cp.tile([121, 128], bf16, tag="y0f")
            nc.vector.tensor_scalar(y0f[0:120, :], c0f[:], 0.0, 6.0,
                                    ALU.max, ALU.min)
            nc.sync.dma_start(y0f[120:121, :], a['WAV'][2:3, 0:128])
            c1f = psC.tile([60, 32], f32, tag="C")
            y0fv = y0f[:].rearrange("p (w k x) -> p w k x", w=8, k=4)
            for dw in range(4):
                nc.tensor.matmul(
                    c1f[:].rearrange("p (w m) -> p w m", m=4),
                    c1w[dw], y0fv[:, :, :, dw],
                    start=(dw == 0), stop=(dw == 3))
            y1f = cp.tile([61, 32], bf16, tag="y1f")
            nc.vector.tensor_scalar(y1f[0:60, :], c1f[:], 0.0, 6.0,
                                    ALU.max, ALU.min)
            nc.sync.dma_start(y1f[60:61, :], a['WAV'][2:3, 0:32])
            c2f = psC.tile([30, 8], f32, tag="C")
            y1fv = y1f[:].rearrange("p (w x) -> p w x", x=4)
            for dw in range(4):
                nc.tensor.matmul(c2f[:], c2w[dw], y1fv[:, :, dw],
                                 start=(dw == 0), stop=(dw == 3))
            nc.vector.tensor_scalar(
                y2a[0:30, :].rearrange("p (d c) -> p d c", d=NC)[:, :, 31],
                c2f[:], 0.0, 6.0, ALU.max, ALU.min)

            c3ps = psC.tile([15, 84], f32, tag="C")
            for dw in range(4):
                rhs = y2a[0:31, dw:dw + 3 * 84].rearrange(
                    "p (n s) -> p n s", s=3)[:, :, 0]
                nc.tensor.matmul(c3ps[:], HEADW[:, 15 * dw:15 * dw + 15], rhs,
                                 start=(dw == 0), stop=(dw == 3))
            y3 = wp.tile([15, 84], bf16, tag="y3")
            nc.vector.tensor_scalar(y3[:], c3ps[:], 0.0, 6.0, ALU.max, ALU.min)
            scr = dp.tile([15, 84], bf16, tag="scr")
            nc.sync.dma_start(scr[:], y3[:])
            y42T = wp.tile([31, 42], bf16, tag="y42T")
            flat = scr[:].rearrange("a b -> (a b)").rearrange(
                "(r m) -> m r", m=30)
            nc.sync.dma_start(y42T[0:30, :], flat)
            nc.sync.dma_start(y42T[30:31, :], a['HONES'][0:1, 0:42])
            p1 = psC.tile([15, 42], f32, tag="C")
            nc.tensor.matmul(p1[:], HEADW[:, 60:75], y42T[:],
                             start=True, stop=True)
            e1 = wp.tile([15, 42], f32, tag="e1")
            nc.scalar.activation(e1[:], p1[:], AF.Exp, scale=-1.0)
            h = wp.tile([16, 42], bf16, tag="h")
            hr = wp.tile([15, 42], f32, tag="hr")
            nc.vector.tensor_scalar(hr[:], e1[:], 1.0, None, ALU.add)
            nc.vector.reciprocal(hr[:], hr[:])
            nc.vector.tensor_copy(h[0:15, :], hr[:])
            nc.sync.dma_start(h[15:16, :], a['HONES'][0:1, 0:42])
            p2 = psC.tile([42, 2], f32, tag="C")
            nc.tensor.matmul(p2[:], h[:], HEADW[0:16, 75:77],
                             start=True, stop=True)
            e2 = wp.tile([42, 2], f32, tag="e2")
            nc.scalar.activation(e2[:], p2[:], AF.Exp, scale=-1.0)
            e2p = wp.tile([42, 2], f32, tag="e2p")
            nc.vector.tensor_scalar(e2p[:], e2[:], 1.0, None, ALU.add)
            o = wp.tile([42, 2], f32, tag="o")
            nc.vector.reciprocal(o[:], e2p[:])
            nc.sync.dma_start(a['out'][:], o[:])
    nc.compile()
    return nc


def _ensure_built():
    global _NC1, _NC2
    if _NC1 is None:
        _NC1 = _build()
    if _NC2 is None:
        _NC2 = _build_head()


def _run_spmd_retry(nc, in_maps, core_ids, trace, trace_cores=None, tries=8):
    import time
    last = None
    for attempt in range(tries):
        try:
            return run_bass_kernel_spmd(nc, in_maps, core_ids, trace=trace,
                                        trace_cores=trace_cores)
        except Exception as e:
            sys.stderr.write(f"WARN: spmd attempt {attempt} failed: {e!r:.300}\n")
            last = e
            time.sleep(2.0 * (attempt + 1))
    raise last


def _run(inputs, trace=False, trace_cores=None):
    _ensure_built()
    c = build_consts(**inputs)
    shared_keys = ['ET', 'EA', 'WAV', 'CW', 'CB', 'CONVW', 'F8', 'F8E5']
    if not USE_DMABCAST:
        shared_keys.append('WAVM')
    shared = {k: c[k] for k in shared_keys}
    in_maps = [{**shared, 'CX': c['CX'][ci]} for ci in range(NC)]
    res1 = _run_spmd_retry(_NC1, in_maps, list(range(NC)), trace, trace_cores)
    gath = np.concatenate(
        [np.asarray(res1.results[ci]['stg']) for ci in range(NC)], 0)
    in2 = [dict(GATH=gath, FXC=c['FXC'], CONVW=c['CONVW'], HEADW=c['HEADW'],
                HONES=c['HONES'], WAV=c['WAV'])]
    res2 = _run_spmd_retry(_NC2, in2, [0], trace)
    out = np.asarray(res2.results[0]['out'], np.float32)
    return out, res1, res2


def kernel(**inputs) -> np.ndarray:
    out, _, _ = _run(inputs, trace=False)
    return out
